# revision 5
# baseline (speedup 1.0000x reference)
"""Trainium2 Bass kernel for nn_BSpanDecoder (scatter_memory problem).

Strategy: data-parallel over batch B across 8 NeuronCores (8 batch rows per
core). On each core, the 16*8=128 (l, b) output rows of probas map exactly
onto the 128 SBUF partitions. The [L,B,V] scatter+softmax is computed
analytically: each row is a constant background exp(-0.01)/Z except at the
<=128 scattered vocab positions, whose values (and Z itself) come from per-row
exp terms. Scattered positions are packed per 1000-wide vocab chunk (PACK
slots per chunk, host-permuted accumulation matrices), scattered into SBUF by
GPSIMD local_scatter (f32 bit pairs as uint16), then one Scalar-engine op
applies y = x*invZ + bg and dense 1MB DMA writes stream the rows to HBM.
"""

import numpy as np

import concourse.bacc as bacc
import concourse.bass as bass
import concourse.mybir as mybir
import concourse.tile as tile
from concourse.bass import IndirectOffsetOnAxis
from concourse.bass_utils import run_bass_kernel_spmd

P = 128
B, T, H, E, V, S, L = 64, 512, 256, 128, 32000, 128, 16
NCORES = 8
BL = B // NCORES           # 8 batch rows per core
RB = T * BL                # 4096 flat (t, b) rows per core
NRC = RB // P              # 32 row chunks of 128
NJ = RB // 512             # 8 energy column chunks of 512
CHUNK_F = 1000             # f32 elements per local_scatter call (num_elems=2000 u16)
NCH = V // CHUNK_F         # 32 scatter chunks
CPD = 2                    # scatter chunks per store unit
E0 = float(np.exp(np.float32(-0.01)))
FDT = mybir.dt.float32
FRT = mybir.dt.float32r

_PROG_CACHE = {}
LAST_RESULTS = None  # for test harness introspection


def _build_program(pack):
    slots = NCH * pack
    nc = bacc.Bacc(
        "TRN2",
        target_bir_lowering=False,
        debug=False,
        enable_asserts=False,
    )

    # ---- DRAM I/O (per-core views; all cores run the same program) ----
    enc_d = nc.dram_tensor("enc", [RB, H], FDT, kind="ExternalInput")
    lastT_d = nc.dram_tensor("lastT", [H, BL], FDT, kind="ExternalInput")
    zidx_d = nc.dram_tensor("zidx", [BL, 1], mybir.dt.int32, kind="ExternalInput")
    embW_d = nc.dram_tensor("embW", [V, E], FDT, kind="ExternalInput")
    w1t_d = nc.dram_tensor("w1t", [H, H], FDT, kind="ExternalInput")
    w2t_d = nc.dram_tensor("w2t", [H, H], FDT, kind="ExternalInput")
    attnb_d = nc.dram_tensor("attnb", [P, 2], FDT, kind="ExternalInput")
    attnv_d = nc.dram_tensor("attnv", [H, 1], FDT, kind="ExternalInput")
    ctrlWT_d = nc.dram_tensor("ctrlWT", [E, E], FDT, kind="ExternalInput")
    ctrlb_d = nc.dram_tensor("ctrlb", [P, 1], FDT, kind="ExternalInput")
    whT_d = nc.dram_tensor("whT", [2 * H + E, H], FDT, kind="ExternalInput")
    bh_d = nc.dram_tensor("bh", [P, 2], FDT, kind="ExternalInput")
    woT_d = nc.dram_tensor("woT", [H, S], FDT, kind="ExternalInput")
    bo_d = nc.dram_tensor("bo", [P, 1], FDT, kind="ExternalInput")
    dmT_d = nc.dram_tensor("dmT", [L * S, slots], FDT, kind="ExternalInput")
    idx16_d = nc.dram_tensor("idx16", [P, NCH * 2 * pack], mybir.dt.int16,
                             kind="ExternalInput")
    ident_d = nc.dram_tensor("ident", [P, P], FDT, kind="ExternalInput")
    e8_d = nc.dram_tensor("e8", [P, BL], FDT, kind="ExternalInput")

    probas_d = nc.dram_tensor("probas", [L * BL, V], FDT, kind="ExternalOutput")
    hidden_d = nc.dram_tensor("hidden", [BL, H], FDT, kind="ExternalOutput")

    with tile.TileContext(nc) as tc:
        with (
            tc.tile_pool(name="const", bufs=1) as cpool,
            tc.tile_pool(name="work", bufs=1) as wpool,
            tc.tile_pool(name="energy", bufs=4) as epool,
            tc.tile_pool(name="psum", bufs=2, space="PSUM") as pspool,
        ):
            # identity first (gates the transpose pipeline), then enc
            ident = cpool.tile([P, P], FDT)
            nc.sync.dma_start(out=ident[:], in_=ident_d.ap())

            with tc.tile_pool(name="enc", bufs=1) as encpool:
                # ---- enc load: [RB, H] as 32 chunks of 128 rows ----
                enc = encpool.tile([P, NRC, H], FDT)  # partition p = row 128c+p
                for g in range(4):
                    nc.sync.dma_start(
                        out=enc[:, g * 8:(g + 1) * 8, :],
                        in_=enc_d.ap().rearrange("(c p) h -> p c h", p=P)[:, g * 8:(g + 1) * 8, :],
                    )

                # ---- weights / constants ----
                e8 = cpool.tile([P, BL], FDT)
                nc.scalar.dma_start(out=e8[:], in_=e8_d.ap())
                w2t = cpool.tile([P, 2, H], FDT, tag="w2t")   # [h' 128][ktile][h 256]
                nc.scalar.dma_start(out=w2t[:], in_=w2t_d.ap().rearrange("(k p) h -> p k h", p=P))
                w1t = cpool.tile([P, 2, H], FDT, tag="w1t")
                nc.scalar.dma_start(out=w1t[:], in_=w1t_d.ap().rearrange("(k p) h -> p k h", p=P))
                attnb = cpool.tile([P, 2], FDT)
                nc.scalar.dma_start(out=attnb[:], in_=attnb_d.ap())
                attnv = cpool.tile([P, 2, 1], FDT, tag="attnv")
                nc.scalar.dma_start(out=attnv[:], in_=attnv_d.ap().rearrange("(k p) o -> p k o", p=P))
                lastT = cpool.tile([P, 2, BL], FDT, tag="lastT")  # [h' 128][ktile][b]
                nc.scalar.dma_start(out=lastT[:], in_=lastT_d.ap().rearrange("(k p) b -> p k b", p=P))
                ctrlWT = cpool.tile([P, E], FDT)
                nc.scalar.dma_start(out=ctrlWT[:], in_=ctrlWT_d.ap())
                ctrlb = cpool.tile([P, 1], FDT)
                nc.scalar.dma_start(out=ctrlb[:], in_=ctrlb_d.ap())
                whT = cpool.tile([P, 5, H], FDT, tag="whT")
                nc.scalar.dma_start(out=whT[:], in_=whT_d.ap().rearrange("(k p) h -> p k h", p=P))
                bh = cpool.tile([P, 2], FDT)
                nc.scalar.dma_start(out=bh[:], in_=bh_d.ap())
                woT = cpool.tile([P, 2, S], FDT, tag="woT")
                nc.scalar.dma_start(out=woT[:], in_=woT_d.ap().rearrange("(k p) s -> p k s", p=P))
                bo = cpool.tile([P, 1], FDT)
                nc.scalar.dma_start(out=bo[:], in_=bo_d.ap())
                zidx = cpool.tile([BL, 1], mybir.dt.int32, tag="zidx")
                nc.scalar.dma_start(out=zidx[:], in_=zidx_d.ap())
                bneg = cpool.tile([P, 1], FDT, tag="bneg")
                nc.vector.memset(bneg[:], -0.01)
                bzero = cpool.tile([1, 1], FDT, tag="bzero")
                nc.vector.memset(bzero[:], 0.0)
                idx16 = cpool.tile([P, NCH * 2 * pack], mybir.dt.int16, tag="idx16")
                nc.scalar.dma_start(out=idx16[:], in_=idx16_d.ap())
                dmT = cpool.tile([P, L, slots], FDT, tag="dmT")
                for g in range(4):
                    nc.sync.dma_start(
                        out=dmT[:, g * 4:(g + 1) * 4, :],
                        in_=dmT_d.ap().rearrange("(l p) k -> p l k", p=P)[:, g * 4:(g + 1) * 4, :],
                    )

                # ---- embedding gather ----
                ez = wpool.tile([BL, E], FDT, tag="ez")
                nc.gpsimd.indirect_dma_start(
                    out=ez[:], out_offset=None,
                    in_=embW_d.ap(),
                    in_offset=IndirectOffsetOnAxis(ap=zidx[:, :1], axis=0),
                )

                # ---- transpose enc -> encT [h' part (2 tiles), (t,b) 4096] ----
                encT = encpool.tile([P, 2, RB], FRT, tag="encT")
                for c in range(NRC):
                    pt = pspool.tile([P, 2 * P], FDT, tag="pt_tr")
                    for hh in range(2):
                        nc.tensor.transpose(
                            out=pt[:, hh * P:(hh + 1) * P],
                            in_=enc[:, c, hh * P:(hh + 1) * P],
                            identity=ident[:],
                        )
                    for hh in range(2):
                        nc.scalar.copy(out=encT[:, hh, c * P:(c + 1) * P],
                                       in_=pt[:, hh * P:(hh + 1) * P])

                # fp32r copies of the energy-matmul weights
                w2tr = cpool.tile([P, 2, H], FRT, tag="w2tr")
                nc.scalar.copy(out=w2tr[:].rearrange("p k h -> p (k h)"),
                               in_=w2t[:].rearrange("p k h -> p (k h)"))
                attnvr = cpool.tile([P, 2, 1], FRT, tag="attnvr")
                nc.scalar.copy(out=attnvr[:].rearrange("p k o -> p (k o)"),
                               in_=attnv[:].rearrange("p k o -> p (k o)"))

                # ---- hproj = W1^T @ lastT, computed once ----
                php = pspool.tile([P, 2 * BL], FDT, tag="pm")
                for hh in range(2):
                    for kk in range(2):
                        nc.tensor.matmul(
                            out=php[:, hh * BL:(hh + 1) * BL],
                            lhsT=w1t[:, kk, hh * P:(hh + 1) * P],
                            rhs=lastT[:, kk, :],
                            start=(kk == 0), stop=(kk == 1),
                        )
                hproj = wpool.tile([P, 2, BL], FDT, tag="hproj")
                nc.vector.tensor_copy(out=hproj[:].rearrange("p k b -> p (k b)"),
                                      in_=php[:])

                # ---- energy + scores (fp32r matmuls) ----
                scores = wpool.tile([1, RB], FDT, tag="scores")
                for j in range(NJ):
                    sl = slice(j * 512, (j + 1) * 512)
                    ps = pspool.tile([1, 512], FDT, tag="ps_scores")
                    for hh in range(2):
                        pe = pspool.tile([P, 512], FDT, tag="pe_energy")
                        for kk in range(2):
                            nc.tensor.matmul(
                                out=pe[:],
                                lhsT=w2tr[:, kk, hh * P:(hh + 1) * P],
                                rhs=encT[:, kk, sl],
                                start=(kk == 0), stop=(kk == 1),
                            )
                        nc.vector.tensor_tensor(
                            out=pe[:], in0=pe[:],
                            in1=hproj[:, hh, :].unsqueeze(1).to_broadcast([P, 64, BL]),
                            op=mybir.AluOpType.add,
                        )
                        een = epool.tile([P, 512], FRT, tag="energy")
                        nc.scalar.activation(
                            out=een[:], in_=pe[:],
                            func=mybir.ActivationFunctionType.Tanh,
                            bias=attnb[:, hh:hh + 1],
                        )
                        nc.tensor.matmul(
                            out=ps[:], lhsT=attnvr[:, hh, :],
                            rhs=een[:],
                            start=(hh == 0), stop=(hh == 1),
                        )
                    nc.scalar.copy(out=scores[:, sl], in_=ps[:])

                # ---- softmax over t (per b): exp + per-b sums (no max-sub;
                #      |scores| <= ||attn_v||_1 so fp32 exp is safe) ----
                nc.scalar.activation(out=scores[:], in_=scores[:],
                                     func=mybir.ActivationFunctionType.Exp,
                                     bias=bzero[:, :1])
                sc_tb = scores[:].rearrange("o (t b) -> o b t", b=BL)
                sm = wpool.tile([1, BL], FDT, tag="sm")
                nc.vector.tensor_reduce(out=sm[:], in_=sc_tb,
                                        axis=mybir.AxisListType.X,
                                        op=mybir.AluOpType.add)
                invs = wpool.tile([1, BL], FDT, tag="invs")
                nc.vector.reciprocal(out=invs[:], in_=sm[:])
                pinv = pspool.tile([BL, 1], FDT, tag="pm")
                nc.tensor.transpose(out=pinv[:], in_=invs[:],
                                    identity=ident[:1, :1])
                invsT = wpool.tile([BL, 1], FDT, tag="invsT")
                nc.vector.tensor_copy(out=invsT[:], in_=pinv[:])

                # ---- wcol[p, c] = exp_scores[128c + p] via PE transposes ----
                pwcol = pspool.tile([P, NRC], FDT, tag="pm")
                for c in range(NRC):
                    nc.tensor.transpose(out=pwcol[:, c:c + 1],
                                        in_=scores[:, c * P:(c + 1) * P],
                                        identity=ident[:1, :1])
                wcol = wpool.tile([P, NRC], FDT, tag="wcol")
                nc.scalar.copy(out=wcol[:], in_=pwcol[:])

                # ---- context = sum_t w[b,t] enc[(t,b), :] (block-diag lhsT),
                #      normalized by invs at the PSUM->SBUF copy ----
                pctx = pspool.tile([BL, H], FDT, tag="pm")
                for c in range(NRC):
                    wf = epool.tile([P, BL], FDT, tag="wf")
                    nc.vector.tensor_scalar(out=wf[:], in0=e8[:],
                                            scalar1=wcol[:, c:c + 1],
                                            scalar2=None, op0=mybir.AluOpType.mult)
                    nc.tensor.matmul(out=pctx[:], lhsT=wf[:], rhs=enc[:, c, :],
                                     start=(c == 0), stop=(c == NRC - 1))
                ctx = wpool.tile([BL, H], FDT, tag="ctx")
                nc.vector.tensor_scalar(out=ctx[:], in0=pctx[:],
                                        scalar1=invsT[:, :1], scalar2=None,
                                        op0=mybir.AluOpType.mult)
                ctxT = wpool.tile([P, 2, BL], FDT, tag="ctxT")
                ptr = pspool.tile([P, 2 * BL], FDT, tag="pm")
                for hh in range(2):
                    nc.tensor.transpose(out=ptr[:, hh * BL:(hh + 1) * BL],
                                        in_=ctx[:, hh * P:(hh + 1) * P],
                                        identity=ident[:BL, :BL])
                nc.vector.tensor_copy(out=ctxT[:].rearrange("p k b -> p (k b)"),
                                      in_=ptr[:])

                # ---- embedding control projection ----
                pez = pspool.tile([P, BL], FDT, tag="pm")
                nc.tensor.transpose(out=pez[:], in_=ez[:], identity=ident[:BL, :BL])
                ezT = wpool.tile([P, BL], FDT, tag="ezT")
                nc.vector.tensor_copy(out=ezT[:], in_=pez[:])
                pctrl = pspool.tile([P, BL], FDT, tag="pm")
                nc.tensor.matmul(out=pctrl[:], lhsT=ctrlWT[:], rhs=ezT[:],
                                 start=True, stop=True)
                ctrl = wpool.tile([P, BL], FDT, tag="ctrl")
                nc.scalar.activation(out=ctrl[:], in_=pctrl[:],
                                     func=mybir.ActivationFunctionType.Identity,
                                     bias=ctrlb[:, :1])
                ezf = wpool.tile([P, BL], FDT, tag="ezf")
                nc.vector.tensor_add(out=ezf[:], in0=ezT[:], in1=ctrl[:])

                # ---- FFNN hidden + gen scores (all [k part, b] layout) ----
                hidT = wpool.tile([P, 2, BL], FDT, tag="hidT")
                for mh in range(2):
                    ph = pspool.tile([P, BL], FDT, tag="pm")
                    rhs_tiles = [ezf[:], ctxT[:, 0, :], ctxT[:, 1, :],
                                 lastT[:, 0, :], lastT[:, 1, :]]
                    for kk, rt in enumerate(rhs_tiles):
                        nc.tensor.matmul(out=ph[:], lhsT=whT[:, kk, mh * P:(mh + 1) * P],
                                         rhs=rt, start=(kk == 0), stop=(kk == 4))
                    nc.scalar.activation(out=hidT[:, mh, :], in_=ph[:],
                                         func=mybir.ActivationFunctionType.Relu,
                                         bias=bh[:, mh:mh + 1])
                pg = pspool.tile([P, BL], FDT, tag="pm")
                for kk in range(2):
                    nc.tensor.matmul(out=pg[:], lhsT=woT[:, kk, :], rhs=hidT[:, kk, :],
                                     start=(kk == 0), stop=(kk == 1))
                genT = wpool.tile([P, BL], FDT, tag="genT")
                nc.scalar.activation(out=genT[:], in_=pg[:],
                                     func=mybir.ActivationFunctionType.Relu,
                                     bias=bo[:, :1])

                # ---- hidden output [b, h] ----
                phid = pspool.tile([BL, H], FDT, tag="pm")
                for mh in range(2):
                    nc.tensor.transpose(out=phid[:, mh * P:(mh + 1) * P],
                                        in_=hidT[:, mh, :], identity=ident[:])
                hidout = wpool.tile([BL, H], FDT, tag="hidout")
                nc.vector.tensor_copy(out=hidout[:], in_=phid[:])
                nc.scalar.dma_start(out=hidden_d.ap(), in_=hidout[:])

                # ---- A_packed = DmPerm @ gen (per l, per slot tile), exp ----
                Es = wpool.tile([P, slots], FDT, tag="Es")
                S1p = wpool.tile([P, slots // P], FDT, tag="S1p")
                for mt in range(slots // P):
                    pA = pspool.tile([P, P], FDT, tag="pm")
                    for l in range(L):
                        nc.tensor.matmul(out=pA[:, l * BL:(l + 1) * BL],
                                         lhsT=dmT[:, l, mt * P:(mt + 1) * P],
                                         rhs=genT[:],
                                         start=True, stop=True)
                    A_T = wpool.tile([P, P], FDT, tag="A_T", bufs=2)
                    nc.vector.tensor_copy(out=A_T[:], in_=pA[:])
                    pAt = pspool.tile([P, P], FDT, tag="pm")
                    nc.tensor.transpose(out=pAt[:], in_=A_T[:], identity=ident[:])
                    nc.scalar.activation(out=Es[:, mt * P:(mt + 1) * P], in_=pAt[:],
                                         func=mybir.ActivationFunctionType.Exp,
                                         bias=bneg[:, :1],
                                         accum_out=S1p[:, mt:mt + 1])
                S1 = wpool.tile([P, 1], FDT, tag="S1")
                nc.vector.tensor_reduce(out=S1[:], in_=S1p[:],
                                        axis=mybir.AxisListType.X,
                                        op=mybir.AluOpType.add)
                Z = wpool.tile([P, 1], FDT, tag="Z")
                nc.vector.tensor_scalar(out=Z[:], in0=S1[:],
                                        scalar1=float((V - slots) * E0),
                                        scalar2=None, op0=mybir.AluOpType.add)
                invZ = wpool.tile([P, 1], FDT, tag="invZ")
                nc.vector.reciprocal(out=invZ[:], in_=Z[:])
                bg = wpool.tile([P, 1], FDT, tag="bg")
                nc.vector.tensor_scalar(out=bg[:], in0=invZ[:], scalar1=E0,
                                        scalar2=None, op0=mybir.AluOpType.mult)
                vals = wpool.tile([P, slots], FDT, tag="vals")
                nc.vector.tensor_scalar(out=vals[:], in0=Es[:], scalar1=E0,
                                        scalar2=None, op0=mybir.AluOpType.subtract)
                d16 = vals[:].bitcast(mybir.dt.uint16)  # [P, 2*slots]

            # ---- scatter + y = x*invZ + bg + store ----
            with (
                tc.tile_pool(name="u16", bufs=3) as upool,
                tc.tile_pool(name="outf", bufs=3) as opool,
            ):
                for u in range(NCH // CPD):
                    u16 = upool.tile([P, CPD * 2 * CHUNK_F], mybir.dt.uint16, tag="u16")
                    for q in range(CPD):
                        c = u * CPD + q
                        nc.gpsimd.local_scatter(
                            out_ap=u16[:, q * 2 * CHUNK_F:(q + 1) * 2 * CHUNK_F],
                            data_ap=d16[:, c * 2 * pack:(c + 1) * 2 * pack],
                            idxs_ap=idx16[:, c * 2 * pack:(c + 1) * 2 * pack],
                            channels=P, num_elems=2 * CHUNK_F, num_idxs=2 * pack,
                        )
                    outf = opool.tile([P, CPD * CHUNK_F], FDT, tag="outf")
                    nc.scalar.activation(out=outf[:], in_=u16[:].bitcast(FDT),
                                         func=mybir.ActivationFunctionType.Identity,
                                         bias=bg[:, :1], scale=invZ[:, :1])
                    eng = nc.sync if (u % 2 == 0) else nc.scalar
                    eng.dma_start(
                        out=probas_d.ap()[:, u * CPD * CHUNK_F:(u + 1) * CPD * CHUNK_F],
                        in_=outf[:],
                    )

    nc.compile()
    return nc


def _get_prog(pack):
    if pack not in _PROG_CACHE:
        _PROG_CACHE[pack] = _build_program(pack)
    return _PROG_CACHE[pack]


def _host_prep(inputs, pack):
    """Build the per-core input maps (host-side sharding + layout prep).
    Returns None if `pack` is too small for this slot_vocab_map."""
    f32 = np.float32
    slots = NCH * pack
    u_enc_out = np.ascontiguousarray(np.asarray(inputs["u_enc_out"], f32))    # [T,B,H]
    last_hidden = np.asarray(inputs["last_hidden"], f32)                      # [1,B,H]
    z_tm1 = np.asarray(inputs["z_tm1"]).astype(np.int32)                      # [1,B]
    cols = np.asarray(inputs["slot_vocab_map"]).astype(np.int64)              # [L,S]
    emb_W = np.ascontiguousarray(np.asarray(inputs["emb_W"], f32))
    emb_ctrl_W = np.asarray(inputs["emb_ctrl_W"], f32)
    emb_ctrl_b = np.asarray(inputs["emb_ctrl_b"], f32)
    attn_W = np.asarray(inputs["attn_W"], f32)
    attn_b = np.asarray(inputs["attn_b"], f32)
    attn_v = np.asarray(inputs["attn_v"], f32)
    Wh = np.asarray(inputs["ffnn_hidden_W"], f32)
    bh = np.asarray(inputs["ffnn_hidden_b"], f32)
    Wo = np.asarray(inputs["ffnn_out_W"], f32)
    bo = np.asarray(inputs["ffnn_out_b"], f32)

    # packed per-chunk slots: masked+permuted accumulation matrices + indices
    dmT = np.zeros((L, S, slots), f32)   # lhsT[s', k]
    idx16 = np.full((P, NCH, 2 * pack), -1, np.int16)
    ar = np.arange(S)
    for l in range(L):
        c = cols[l]
        D = c[:, None] == c[None, :]
        first = D.argmax(axis=1) == ar          # first occurrence of each value
        Dm = (D & first[:, None]).astype(f32)   # [slot s, s']
        ch_of = c // CHUNK_F
        for ch in range(NCH):
            sel = ar[first & (ch_of == ch)]
            if len(sel) > pack:
                return None                     # pack too small; caller rebuilds
            for j, s in enumerate(sel):
                k = ch * pack + j
                dmT[l, :, k] = Dm[s, :]
                off = int(c[s]) % CHUNK_F
                idx16[l * BL:(l + 1) * BL, ch, 2 * j] = 2 * off
                idx16[l * BL:(l + 1) * BL, ch, 2 * j + 1] = 2 * off + 1
    dmT = dmT.reshape(L * S, slots)
    idx16 = idx16.reshape(P, NCH * 2 * pack)

    ident = np.eye(P, dtype=f32)
    e8 = (np.arange(P)[:, None] % BL == np.arange(BL)[None, :]).astype(f32)

    shared = {
        "embW": emb_W,
        "w1t": np.ascontiguousarray(attn_W[:, :H].T),
        "w2t": np.ascontiguousarray(attn_W[:, H:].T),
        "attnb": np.ascontiguousarray(attn_b.reshape(2, P).T),
        "attnv": np.ascontiguousarray(attn_v.reshape(H, 1)),
        "ctrlWT": np.ascontiguousarray(emb_ctrl_W.T),
        "ctrlb": np.ascontiguousarray(emb_ctrl_b.reshape(P, 1)),
        "whT": np.ascontiguousarray(Wh.T),
        "bh": np.ascontiguousarray(bh.reshape(2, P).T),
        "woT": np.ascontiguousarray(Wo.T),
        "bo": np.ascontiguousarray(bo.reshape(P, 1)),
        "dmT": dmT,
        "idx16": idx16,
        "ident": ident,
        "e8": e8,
    }
    in_maps = []
    for k in range(NCORES):
        bs = slice(k * BL, (k + 1) * BL)
        m = dict(shared)
        m["enc"] = np.ascontiguousarray(u_enc_out[:, bs, :].reshape(RB, H))
        m["lastT"] = np.ascontiguousarray(last_hidden[0, bs, :].T)
        m["zidx"] = np.ascontiguousarray(z_tm1[0, bs].reshape(BL, 1))
        in_maps.append(m)
    return in_maps


def kernel(**inputs):
    global LAST_RESULTS
    pack = 16
    in_maps = _host_prep(inputs, pack)
    while in_maps is None:  # extremely unlikely: a vocab chunk holds >pack slots
        pack *= 2
        in_maps = _host_prep(inputs, pack)
    nc = _get_prog(pack)
    res = run_bass_kernel_spmd(nc, in_maps, list(range(NCORES)))
    LAST_RESULTS = res
    hidden = np.empty((1, B, H), np.float32)
    probas = np.empty((L, B, V), np.float32)
    for k in range(NCORES):
        bs = slice(k * BL, (k + 1) * BL)
        hidden[0, bs, :] = res.results[k]["hidden"]
        probas[:, bs, :] = res.results[k]["probas"].reshape(L, BL, V)
    return hidden, probas
